# revision 32
# baseline (speedup 1.0000x reference)
"""DrugGNN segment-mean + linear embed, v4: single-path all-PE design.

Per core: 2048 segs = 16 groups x 128 segs = 64 blocks x 32 segs.
  - Host pads every segment count to a multiple of 8 ("slot rows" of 8
    nodes), snake-packs segments into 512 (core, block) bins of exactly 32
    segments (segments assigned to arbitrary cores/blocks; rows un-permuted
    on the host afterwards) so every block fits T*128 = 512 slot rows, and
    sigma-delta quantizes x on the fp8-e3m4 grid (error feedback keeps
    per-segment sums exact to ~1 quant step).
  - Block slab [128p, T*512] fp8: Q-group g = cols [g*512,(g+1)*512); its
    8 tiles of 64 channels share ONE onehot pattern (row -> local seg), so
    each Q-group is a single matmul: lhsT = onehot [128,32] fp8, rhs =
    slab [128,512], out = acc[strip:strip+32] with stride-0 free AP
    [(0,8),(1,64)] accumulating all 8 tiles into the same PSUM columns
    (~125ns per 1024 nodes warm). tile_position=(0,strip) stacks the 4
    blocks of a group into one [128,64] accumulator.
  - Onehots built on DVE: is_equal(iota, srel bcast [(1,T),(0,32)]);
    iota + identity built on device via gpsimd iota/affine_select.
  - x streamed as 2-block 512KB DMAs (4KB lines) rotating over the three
    DMA queues (sync/scalar/gpsimd, ~115GB/s each); small tensors (srel,
    scale) are partition-split 3 ways because sub-1KB-line DMAs are
    per-line-overhead-bound. PE warmup dummies bridge the HAM clock ramp.
  - Epilogue per group: DVE 1/cnt scale -> fp16 means + ones col, PE
    transpose, fp16 GEMM (lhsT=[weight.T;bias]) emitting [out_ch, segs]
    into one SBUF buffer; single partition-split output DMA at the end.
"""
import numpy as np

N_NODES = 2_000_000
IN_CH = 64
OUT_CH = 128
NUM_GRAPHS = 16384
N_CORES = 8
P = 128
SEGS_PER_CORE = NUM_GRAPHS // N_CORES   # 2048
NGROUP = SEGS_PER_CORE // P             # 16 groups of 128 segs
NBLK = 4 * NGROUP                       # 64 blocks of 32 segs per core
NBIN = N_CORES * NBLK                   # 512 bins globally
LOOKAHEAD = 24                          # blocks of produce-ahead
WARMUP = 56                             # PE warmup dummy matmuls
DMA_BLOCKS = 2                          # blocks per slab DMA (1 or 2)
OUT_MODE = 1                            # 1=single end DMA, 2=per-group DMAs
OUT64 = False                           # 64-partition output layout (tested: no gain)
DEFER_SMALL = False                     # tested: deferring is ~2.7us worse

TRACE = False
LAST_RESULT = None
_BUILD_CACHE = {}


def _build(T):
    from contextlib import ExitStack
    import concourse.bass as bass
    import concourse.bacc as bacc
    import concourse.tile as tile
    from concourse import mybir

    nc = bacc.Bacc("TRN2", target_bir_lowering=False, debug=False,
                   num_devices=N_CORES)
    dt = mybir.dt
    SREL_B = NBLK * T * 2               # srel bytes at head of xq
    xq = nc.dram_tensor("xq", [P, SREL_B + NBLK * T * 512], dt.float8e3,
                        kind="ExternalInput").ap()
    wb = nc.dram_tensor("wb", [IN_CH + 1, OUT_CH], dt.float16,
                        kind="ExternalInput").ap()
    scale = nc.dram_tensor("scale", [P, NGROUP], dt.float32,
                           kind="ExternalInput").ap()
    if OUT64:
        out = nc.dram_tensor("out", [OUT_CH // 2, 2 * SEGS_PER_CORE],
                             dt.float16, kind="ExternalOutput").ap()
    else:
        out = nc.dram_tensor("out", [OUT_CH, SEGS_PER_CORE], dt.float16,
                             kind="ExternalOutput").ap()

    def ap3(t_, off, d1, d2):
        return bass.AP(tensor=t_.tensor, offset=t_.offset + off,
                       ap=[t_.ap[0], d1, d2])

    def ap4(t_, off, d1, d2, d3):
        return bass.AP(tensor=t_.tensor, offset=t_.offset + off,
                       ap=[t_.ap[0], d1, d2, d3])

    with tile.TileContext(nc) as tc, ExitStack() as ctx:
        singles = ctx.enter_context(tc.tile_pool(name="singles", bufs=1))
        slabs = ctx.enter_context(
            tc.tile_pool(name="slabs", bufs=LOOKAHEAD // DMA_BLOCKS + 2))
        ohpool = ctx.enter_context(
            tc.tile_pool(name="ohpool", bufs=LOOKAHEAD + 3))
        meanpool = ctx.enter_context(tc.tile_pool(name="meanpool", bufs=2))
        sbtpool = ctx.enter_context(tc.tile_pool(name="sbtpool", bufs=2))
        outpool = ctx.enter_context(tc.tile_pool(name="outpool", bufs=2))
        psum_acc = ctx.enter_context(
            tc.tile_pool(name="psum_acc", bufs=3, space="PSUM"))
        psum_t = ctx.enter_context(
            tc.tile_pool(name="psum_t", bufs=2, space="PSUM"))
        psum_o = ctx.enter_context(
            tc.tile_pool(name="psum_o", bufs=2, space="PSUM"))

        accs = {}

        def epilogue(g):
            acc = accs.pop(g)
            means = meanpool.tile([P, IN_CH + 1], dt.float16)
            nc.vector.tensor_scalar_mul(means[:, 0:IN_CH], acc,
                                        scale_sb[:, g:g + 1])
            nc.gpsimd.memset(means[:, IN_CH:IN_CH + 1], 1.0)
            pt = psum_t.tile([IN_CH + 1, P], dt.float16)
            nc.tensor.transpose(pt, means, ident_sb)
            sbt = sbtpool.tile([IN_CH + 1, P], dt.float16)
            nc.vector.tensor_copy(sbt, pt)
            if OUT64:
                po = psum_o.tile([OUT_CH // 2, 2 * P], dt.float32)
                nc.tensor.matmul(po[:, 0:P], lhsT=wb_sb[:, 0:OUT_CH // 2],
                                 rhs=sbt, start=True, stop=True)
                nc.tensor.matmul(po[:, P:2 * P], lhsT=wb_sb[:, OUT_CH // 2:],
                                 rhs=sbt, start=True, stop=True)
                nc.vector.tensor_copy(
                    osb_all[:, g * 2 * P:(g + 1) * 2 * P], po)
            else:
                po = psum_o.tile([OUT_CH, P], dt.float32)
                nc.tensor.matmul(po, lhsT=wb_sb, rhs=sbt,
                                 start=True, stop=True)
                nc.vector.tensor_copy(osb_all[:, g * P:(g + 1) * P], po)

        produced = {}
        oh_made = {}
        dma_i = 0
        GW = 4 * T * 512                # group width in slab cols

        BW_ = DMA_BLOCKS * T * 512      # slab DMA width

        def produce(sb):
            nonlocal dma_i
            if DMA_BLOCKS * sb >= NBLK:
                return
            ring = (nc.sync, nc.scalar, nc.gpsimd)[dma_i % 3]
            dma_i += 1
            xs = slabs.tile([P, BW_], dt.float8e3, name="xs")
            ring.dma_start(xs, xq[:, SREL_B + sb * BW_:
                                  SREL_B + (sb + 1) * BW_])
            produced[sb] = xs

        def make_oh(b):
            if b >= NBLK:
                return
            oh = ohpool.tile([P, T * 32], dt.float8e3, name="oh")
            nc.vector.tensor_tensor(
                oh, iota_sb, ap3(srel_sb, b * T, [1, T], [0, 32]),
                mybir.AluOpType.is_equal)
            oh_made[b] = oh

        def consume(b):
            g_idx = b // 4
            strip = 32 * (b % 4)
            u = b // DMA_BLOCKS
            xs = produced[u] if b % DMA_BLOCKS != DMA_BLOCKS - 1 \
                else produced.pop(u)
            off = (b % DMA_BLOCKS) * T * 512
            oh = oh_made.pop(b)
            acc = accs[g_idx]
            sl = acc[strip:strip + 32, :]
            dst = bass.AP(tensor=sl.tensor, offset=sl.offset,
                          ap=[sl.ap[0], [0, 8], [1, IN_CH]])
            for g in range(T):
                nc.tensor.matmul(
                    dst,
                    lhsT=oh[:, g * 32:(g + 1) * 32],
                    rhs=xs[:, off + g * 512:off + (g + 1) * 512],
                    start=(g == 0), stop=(g == T - 1),
                    tile_position=(0, strip))

        # PE warmup: dummy matmuls so HAM unthrottles before real work
        # arrives; deps only on a memset so they start immediately.
        wa = singles.tile([P, IN_CH], dt.float16, name="wa")
        nc.vector.memset(wa, 0.0)
        pw = psum_t.tile([32, IN_CH], dt.float32, name="pt")
        for r in range(WARMUP):
            nc.tensor.matmul(pw, lhsT=wa[:, 0:32], rhs=wa,
                             start=True, stop=True)

        # first DMA carries srel + pair 0 in one transfer (srel bytes are
        # embedded at the head of xq; read back via bitcast)
        xs0 = singles.tile([P, SREL_B + BW_], dt.float8e3, name="xs0")
        nc.sync.dma_start(xs0, xq[:, 0:SREL_B + BW_])
        dma_i = 1
        produced[0] = xs0[:, SREL_B:]
        srel_sb = xs0[:, 0:SREL_B].bitcast(dt.bfloat16)
        produce(1)
        produce(2)
        scale_sb = singles.tile([P, NGROUP], dt.float32, name="scale")
        wb_sb = singles.tile([IN_CH + 1, OUT_CH], dt.float16, name="wb")
        if not DEFER_SMALL:
            for i, ring in enumerate((nc.sync, nc.scalar, nc.gpsimd)):
                p0, p1 = (P * i) // 3, (P * (i + 1)) // 3
                ring.dma_start(scale_sb[p0:p1, :], scale[p0:p1, :])
            nc.scalar.dma_start(wb_sb, wb)
        # iota + identity built on device (gpsimd), no DMA at all
        iota_sb = singles.tile([P, T * 32], dt.bfloat16, name="iota")
        nc.gpsimd.iota(iota_sb, pattern=[[0, T], [1, 32]], base=0,
                       channel_multiplier=0,
                       allow_small_or_imprecise_dtypes=True)
        ident_sb = singles.tile([P, P], dt.float16, name="ident")
        nc.gpsimd.memset(ident_sb, 1.0)
        nc.gpsimd.affine_select(ident_sb, ident_sb, pattern=[[1, P]],
                                compare_op=mybir.AluOpType.is_equal,
                                fill=0.0, base=0, channel_multiplier=-1)
        if OUT64:
            osb_all = singles.tile([OUT_CH // 2, 2 * SEGS_PER_CORE],
                                   dt.float16, name="osb")
        else:
            osb_all = singles.tile([OUT_CH, SEGS_PER_CORE], dt.float16,
                                   name="osb")
        for sb in range(3, LOOKAHEAD // DMA_BLOCKS):
            produce(sb)
        if DEFER_SMALL:
            for i, ring in enumerate((nc.sync, nc.scalar, nc.gpsimd)):
                p0, p1 = (P * i) // 3, (P * (i + 1)) // 3
                ring.dma_start(scale_sb[p0:p1, :], scale[p0:p1, :])
            nc.scalar.dma_start(wb_sb, wb)
        for b in range(min(LOOKAHEAD, NBLK)):
            make_oh(b)
        for g_idx in range(NGROUP):
            accs[g_idx] = psum_acc.tile([P, IN_CH], dt.float32, name="acc")
            for j in range(4):
                b = 4 * g_idx + j
                consume(b)
                if (b + LOOKAHEAD) % DMA_BLOCKS == 0:
                    produce((b + LOOKAHEAD) // DMA_BLOCKS)
                make_oh(b + LOOKAHEAD)
            if g_idx >= 1:
                epilogue(g_idx - 1)
        epilogue(NGROUP - 1)
        if OUT_MODE == 1:
            nc.sync.dma_start(out, osb_all)
    nc.compile()
    return nc


def _sigma_delta_fp8(xpad, valid, qdtype):
    """Native-grid error-feedback quantization along axis 1."""
    S, L, F = xpad.shape
    q = np.zeros((S, L, F), qdtype)
    delta = np.zeros((S, F), np.float32)
    for j in range(L):
        m = valid[:, j][:, None]
        a = xpad[:, j, :] + delta
        qj = a.astype(qdtype)
        qf = qj.astype(np.float32)
        q[:, j, :] = np.where(m, qj, np.zeros((), qdtype))
        delta = np.where(m, a - qf, delta)
    return q


def _ensure_ntff_hook():
    import sys
    import types
    try:
        import antenv.axon_hooks  # noqa: F401
        return
    except ImportError:
        pass
    import antenv
    mod = types.ModuleType("antenv.axon_hooks")
    holder = {"h": None}
    mod.set_axon_ntff_profile_hook = lambda h: holder.__setitem__("h", h)
    mod.get_axon_ntff_profile_hook = lambda: holder["h"]
    sys.modules["antenv.axon_hooks"] = mod
    antenv.axon_hooks = mod
    try:
        from trn_agent_boot.trn_boot import _ntff_profile_via_ctypes
        mod.set_axon_ntff_profile_hook(
            _ntff_profile_via_ctypes("/opt/axon/libaxon_pjrt.so"))
    except Exception as e:
        print(f"ntff hook unavailable: {e}")


def kernel(x, segment_ids, weight, bias, num_graphs):
    global LAST_RESULT
    import ml_dtypes
    from concourse import bass_utils

    if TRACE:
        _ensure_ntff_hook()

    f8e4 = ml_dtypes.float8_e3m4
    bf16 = ml_dtypes.bfloat16
    x = np.asarray(x, dtype=np.float32)
    seg = np.asarray(segment_ids).astype(np.int64)
    weight = np.asarray(weight, dtype=np.float32)
    bias = np.asarray(bias, dtype=np.float32)
    G = int(num_graphs)
    assert G == NUM_GRAPHS and x.shape == (N_NODES, IN_CH)

    bounds = np.searchsorted(seg, np.arange(G + 1))
    cnts = np.diff(bounds).astype(np.int64)
    m = (cnts + 7) // 8                      # slot rows per seg

    # ---- snake-pack segments into 512 bins of exactly 32 segs ----
    order = np.argsort(-m, kind="stable")
    bin_of_seg = np.empty(G, np.int64)
    local_of_seg = np.empty(G, np.int64)
    fwd = np.arange(NBIN)
    rev = fwd[::-1]
    for r in range(G // NBIN):               # 32 rounds
        rowsegs = order[r * NBIN:(r + 1) * NBIN]
        bins = fwd if r % 2 == 0 else rev
        bin_of_seg[rowsegs] = bins
        local_of_seg[rowsegs] = r
    R = np.zeros(NBIN, np.int64)
    np.add.at(R, bin_of_seg, m)
    T = int(np.ceil(R.max() / P))
    assert T * P >= R.max()

    # per-seg starting slot row within its block (assignment order per bin)
    row_start = np.zeros(G, np.int64)
    base = np.zeros(NBIN, np.int64)
    for r in range(G // NBIN):
        rowsegs = order[r * NBIN:(r + 1) * NBIN]
        b = bin_of_seg[rowsegs]
        row_start[rowsegs] = base[b]
        base[b] += m[rowsegs]

    # ---- sigma-delta quantize on e4m3 grid ----
    L = int(m.max() * 8)
    idx_in_seg = np.arange(N_NODES) - bounds[seg]
    xpad = np.zeros((G, L, IN_CH), np.float32)
    vpad = np.zeros((G, L), bool)
    xpad[seg, idx_in_seg] = x
    vpad[seg, idx_in_seg] = True
    q = _sigma_delta_fp8(xpad, vpad, f8e4)   # [G, L, F]
    del xpad, vpad
    q = q.reshape(G, L // 8, 8, IN_CH)

    # ---- scatter into per-core slabs ----
    # per slot row: seg, row index within block
    seg_rep = np.repeat(np.arange(G), m)                    # [Rtot]
    csum = np.concatenate([[0], np.cumsum(m)])
    r_in_seg = np.arange(len(seg_rep)) - csum[seg_rep]      # [Rtot]
    row_blk = row_start[seg_rep] + r_in_seg                 # block row
    bin_r = bin_of_seg[seg_rep]
    core_r = bin_r // NBLK
    blk_r = bin_r % NBLK
    g_r = row_blk // P
    p_r = row_blk % P

    SREL_B = NBLK * T * 2
    xq_all = np.zeros((N_CORES, P, SREL_B + NBLK * T * 512), f8e4)
    vals = q[seg_rep, r_in_seg]                             # [Rtot, 8, F]
    cols = (SREL_B + blk_r * (T * 512) + g_r * 512)[:, None, None] + \
        (np.arange(8) * IN_CH)[None, :, None] + \
        np.arange(IN_CH)[None, None, :]
    xq_all[core_r[:, None, None], p_r[:, None, None], cols] = vals
    del q, vals, cols

    srel_all = np.full((N_CORES, P, NBLK * T), -1.0, np.float32)
    srel_all[core_r, p_r, blk_r * T + g_r] = local_of_seg[seg_rep]
    xq_all[:, :, 0:SREL_B] = srel_all.astype(bf16).view(np.uint8).view(f8e4)

    # ---- epilogue scale + output permutation ----
    # device row (core, grp*128 + p) holds seg with bin=core*NBLK+grp*4+p//32,
    # local=p%32
    grp = np.arange(SEGS_PER_CORE) // P
    p_of = np.arange(SEGS_PER_CORE) % P
    seg_at = np.empty((N_CORES, SEGS_PER_CORE), np.int64)
    inv = np.empty(G, np.int64)
    inv[bin_of_seg * 32 + local_of_seg] = np.arange(G)
    for c in range(N_CORES):
        bins_ = c * NBLK + grp * 4 + p_of // 32
        seg_at[c] = inv[bins_ * 32 + p_of % 32]
    sc = 1.0 / np.maximum(cnts, 1).astype(np.float32)[seg_at]  # [C, 2048]
    scale_all = np.ascontiguousarray(
        sc.reshape(N_CORES, NGROUP, P).transpose(0, 2, 1)).astype(np.float32)

    wb = np.concatenate([weight.T, bias[None]], axis=0).astype(np.float16)

    bkey = (T, OUT64, DEFER_SMALL, WARMUP)
    if bkey not in _BUILD_CACHE:
        _BUILD_CACHE[bkey] = _build(T)
    nc = _BUILD_CACHE[bkey]

    in_maps = [
        dict(xq=xq_all[c], wb=wb, scale=scale_all[c])
        for c in range(N_CORES)
    ]
    res = bass_utils.run_bass_kernel_spmd(
        nc, in_maps, core_ids=list(range(N_CORES)), trace=TRACE)
    LAST_RESULT = res
    if OUT64:
        dev = np.concatenate(
            [res.results[c]["out"].reshape(OUT_CH // 2, NGROUP, 2, P)
             .transpose(2, 0, 1, 3).reshape(OUT_CH, SEGS_PER_CORE).T
             for c in range(N_CORES)], axis=0)
    else:
        dev = np.concatenate(
            [res.results[c]["out"].T for c in range(N_CORES)], axis=0)
    out_full = np.empty((G, OUT_CH), np.float32)
    out_full[seg_at.reshape(-1)] = dev.astype(np.float32)
    return out_full


# revision 35
# speedup vs baseline: 1.0913x; 1.0913x over previous
"""DrugGNN segment-mean + linear embed, v4: single-path all-PE design.

Per core: 2048 segs = 16 groups x 128 segs = 64 blocks x 32 segs.
  - Host pads every segment count to a multiple of 8 ("slot rows" of 8
    nodes), snake-packs segments into 512 (core, block) bins of exactly 32
    segments (segments assigned to arbitrary cores/blocks; rows un-permuted
    on the host afterwards) so every block fits T*128 = 512 slot rows, and
    sigma-delta quantizes x on the fp8-e3m4 grid (error feedback keeps
    per-segment sums exact to ~1 quant step).
  - Block slab [128p, T*512] fp8: Q-group g = cols [g*512,(g+1)*512); its
    8 tiles of 64 channels share ONE onehot pattern (row -> local seg), so
    each Q-group is a single matmul: lhsT = onehot [128,32] fp8, rhs =
    slab [128,512], out = acc[strip:strip+32] with stride-0 free AP
    [(0,8),(1,64)] accumulating all 8 tiles into the same PSUM columns
    (~125ns per 1024 nodes warm). tile_position=(0,strip) stacks the 4
    blocks of a group into one [128,64] accumulator.
  - Onehots built on DVE: is_equal(iota, srel bcast [(1,T),(0,32)]);
    iota + identity built on device via gpsimd iota/affine_select.
  - x streamed as 2-block 512KB DMAs (4KB lines) rotating over the three
    DMA queues (sync/scalar/gpsimd, ~115GB/s each); small tensors (srel,
    scale) are partition-split 3 ways because sub-1KB-line DMAs are
    per-line-overhead-bound. PE warmup dummies bridge the HAM clock ramp.
  - Epilogue per group: DVE 1/cnt scale -> fp16 means + ones col, PE
    transpose, fp16 GEMM (lhsT=[weight.T;bias]) emitting [out_ch, segs]
    into one SBUF buffer; single partition-split output DMA at the end.
"""
import numpy as np

N_NODES = 2_000_000
IN_CH = 64
OUT_CH = 128
NUM_GRAPHS = 16384
N_CORES = 8
P = 128
SEGS_PER_CORE = NUM_GRAPHS // N_CORES   # 2048
NGROUP = SEGS_PER_CORE // P             # 16 groups of 128 segs
NBLK = 4 * NGROUP                       # 64 blocks of 32 segs per core
NBIN = N_CORES * NBLK                   # 512 bins globally
LOOKAHEAD = 24                          # blocks of produce-ahead
WARMUP = 56                             # PE warmup dummy matmuls
DMA_BLOCKS = 2                          # blocks per slab DMA (1 or 2)
OUT_MODE = 1                            # 1=single end DMA, 2=per-group DMAs
OUT64 = False                           # 64-partition output layout (tested: no gain)
DEFER_SMALL = False                     # tested: deferring is ~2.7us worse

TRACE = False
LAST_RESULT = None
_BUILD_CACHE = {}


def _build(T):
    from contextlib import ExitStack
    import concourse.bass as bass
    import concourse.bacc as bacc
    import concourse.tile as tile
    from concourse import mybir

    nc = bacc.Bacc("TRN2", target_bir_lowering=False, debug=False,
                   num_devices=N_CORES)
    dt = mybir.dt
    HEAD = 1024                         # srel/scale/wb bytes at head of xq
    xq = nc.dram_tensor("xq", [P, HEAD + NBLK * T * 512], dt.float8e3,
                        kind="ExternalInput").ap()
    if OUT64:
        out = nc.dram_tensor("out", [OUT_CH // 2, 2 * SEGS_PER_CORE],
                             dt.float16, kind="ExternalOutput").ap()
    else:
        out = nc.dram_tensor("out", [OUT_CH, SEGS_PER_CORE], dt.float16,
                             kind="ExternalOutput").ap()

    def ap3(t_, off, d1, d2):
        return bass.AP(tensor=t_.tensor, offset=t_.offset + off,
                       ap=[t_.ap[0], d1, d2])

    def ap4(t_, off, d1, d2, d3):
        return bass.AP(tensor=t_.tensor, offset=t_.offset + off,
                       ap=[t_.ap[0], d1, d2, d3])

    with tile.TileContext(nc) as tc, ExitStack() as ctx:
        singles = ctx.enter_context(tc.tile_pool(name="singles", bufs=1))
        slabs = ctx.enter_context(
            tc.tile_pool(name="slabs", bufs=LOOKAHEAD // DMA_BLOCKS + 2))
        ohpool = ctx.enter_context(
            tc.tile_pool(name="ohpool", bufs=LOOKAHEAD + 3))
        meanpool = ctx.enter_context(tc.tile_pool(name="meanpool", bufs=2))
        sbtpool = ctx.enter_context(tc.tile_pool(name="sbtpool", bufs=2))
        outpool = ctx.enter_context(tc.tile_pool(name="outpool", bufs=2))
        psum_acc = ctx.enter_context(
            tc.tile_pool(name="psum_acc", bufs=3, space="PSUM"))
        psum_t = ctx.enter_context(
            tc.tile_pool(name="psum_t", bufs=2, space="PSUM"))
        psum_o = ctx.enter_context(
            tc.tile_pool(name="psum_o", bufs=2, space="PSUM"))

        accs = {}

        def epilogue(g):
            acc = accs.pop(g)
            means = meanpool.tile([P, IN_CH + 1], dt.float16)
            nc.vector.tensor_scalar_mul(means[:, 0:IN_CH], acc,
                                        scale_sb[:, g:g + 1])
            nc.gpsimd.memset(means[:, IN_CH:IN_CH + 1], 1.0)
            pt = psum_t.tile([IN_CH + 1, P], dt.float16)
            nc.tensor.transpose(pt, means, ident_sb)
            sbt = sbtpool.tile([IN_CH + 1, P], dt.float16)
            nc.vector.tensor_copy(sbt, pt)
            if OUT64:
                po = psum_o.tile([OUT_CH // 2, 2 * P], dt.float32)
                nc.tensor.matmul(po[:, 0:P], lhsT=wb_sb[:, 0:OUT_CH // 2],
                                 rhs=sbt, start=True, stop=True)
                nc.tensor.matmul(po[:, P:2 * P], lhsT=wb_sb[:, OUT_CH // 2:],
                                 rhs=sbt, start=True, stop=True)
                nc.vector.tensor_copy(
                    osb_all[:, g * 2 * P:(g + 1) * 2 * P], po)
            else:
                po = psum_o.tile([OUT_CH, P], dt.float32)
                nc.tensor.matmul(po, lhsT=wb_sb, rhs=sbt,
                                 start=True, stop=True)
                nc.vector.tensor_copy(osb_all[:, g * P:(g + 1) * P], po)

        produced = {}
        oh_made = {}
        dma_i = 0
        GW = 4 * T * 512                # group width in slab cols

        BW_ = DMA_BLOCKS * T * 512      # slab DMA width

        def produce(sb):
            nonlocal dma_i
            if DMA_BLOCKS * sb >= NBLK:
                return
            ring = (nc.sync, nc.scalar, nc.gpsimd)[dma_i % 3]
            dma_i += 1
            xs = slabs.tile([P, BW_], dt.float8e3, name="xs")
            ring.dma_start(xs, xq[:, HEAD + sb * BW_:
                                  HEAD + (sb + 1) * BW_])
            produced[sb] = xs

        def make_oh(b):
            if b >= NBLK:
                return
            oh = ohpool.tile([P, T * 32], dt.float8e3, name="oh")
            nc.vector.tensor_tensor(
                oh, iota_sb, ap3(srel_sb, b * T, [1, T], [0, 32]),
                mybir.AluOpType.is_equal)
            oh_made[b] = oh

        def consume(b):
            g_idx = b // 4
            strip = 32 * (b % 4)
            u = b // DMA_BLOCKS
            xs = produced[u] if b % DMA_BLOCKS != DMA_BLOCKS - 1 \
                else produced.pop(u)
            off = (b % DMA_BLOCKS) * T * 512
            oh = oh_made.pop(b)
            acc = accs[g_idx]
            sl = acc[strip:strip + 32, :]
            dst = bass.AP(tensor=sl.tensor, offset=sl.offset,
                          ap=[sl.ap[0], [0, 8], [1, IN_CH]])
            for g in range(T):
                nc.tensor.matmul(
                    dst,
                    lhsT=oh[:, g * 32:(g + 1) * 32],
                    rhs=xs[:, off + g * 512:off + (g + 1) * 512],
                    start=(g == 0), stop=(g == T - 1),
                    tile_position=(0, strip))

        # PE warmup: dummy matmuls so HAM unthrottles before real work
        # arrives; deps only on a memset so they start immediately.
        wa = singles.tile([P, IN_CH], dt.float16, name="wa")
        nc.vector.memset(wa, 0.0)
        pw = psum_t.tile([32, IN_CH], dt.float32, name="pt")
        for r in range(WARMUP):
            nc.tensor.matmul(pw, lhsT=wa[:, 0:32], rhs=wa,
                             start=True, stop=True)

        # first DMA carries srel + scale + wb + pair 0 in one transfer
        # (bytes embedded at the head of xq; read back via bitcast views)
        xs0 = singles.tile([P, HEAD + BW_], dt.float8e3, name="xs0")
        nc.sync.dma_start(xs0, xq[:, 0:HEAD + BW_])
        dma_i = 1
        produced[0] = xs0[:, HEAD:]
        srel_sb = xs0[:, 0:NBLK * T * 2].bitcast(dt.bfloat16)
        scale_sb = xs0[:, 512:512 + 4 * NGROUP].bitcast(dt.float32)
        wb_sb = xs0[0:IN_CH + 1, 576:576 + 2 * OUT_CH].bitcast(dt.float16)
        produce(1)
        produce(2)
        # iota + identity built on device (gpsimd), no DMA at all
        iota_sb = singles.tile([P, T * 32], dt.bfloat16, name="iota")
        nc.gpsimd.iota(iota_sb, pattern=[[0, T], [1, 32]], base=0,
                       channel_multiplier=0,
                       allow_small_or_imprecise_dtypes=True)
        ident_sb = singles.tile([P, P], dt.float16, name="ident")
        nc.gpsimd.memset(ident_sb, 1.0)
        nc.gpsimd.affine_select(ident_sb, ident_sb, pattern=[[1, P]],
                                compare_op=mybir.AluOpType.is_equal,
                                fill=0.0, base=0, channel_multiplier=-1)
        if OUT64:
            osb_all = singles.tile([OUT_CH // 2, 2 * SEGS_PER_CORE],
                                   dt.float16, name="osb")
        else:
            osb_all = singles.tile([OUT_CH, SEGS_PER_CORE], dt.float16,
                                   name="osb")
        for sb in range(3, LOOKAHEAD // DMA_BLOCKS):
            produce(sb)
        for b in range(min(LOOKAHEAD, NBLK)):
            make_oh(b)
        for g_idx in range(NGROUP):
            accs[g_idx] = psum_acc.tile([P, IN_CH], dt.float32, name="acc")
            for j in range(4):
                b = 4 * g_idx + j
                consume(b)
                if (b + LOOKAHEAD) % DMA_BLOCKS == 0:
                    produce((b + LOOKAHEAD) // DMA_BLOCKS)
                make_oh(b + LOOKAHEAD)
            if g_idx >= 1:
                epilogue(g_idx - 1)
        epilogue(NGROUP - 1)
        if OUT_MODE == 1:
            nc.sync.dma_start(out, osb_all)
    nc.compile()
    return nc


def _sigma_delta_fp8(xpad, valid, qdtype):
    """Native-grid error-feedback quantization along axis 1."""
    S, L, F = xpad.shape
    q = np.zeros((S, L, F), qdtype)
    delta = np.zeros((S, F), np.float32)
    for j in range(L):
        m = valid[:, j][:, None]
        a = xpad[:, j, :] + delta
        qj = a.astype(qdtype)
        qf = qj.astype(np.float32)
        q[:, j, :] = np.where(m, qj, np.zeros((), qdtype))
        delta = np.where(m, a - qf, delta)
    return q


def _ensure_ntff_hook():
    import sys
    import types
    try:
        import antenv.axon_hooks  # noqa: F401
        return
    except ImportError:
        pass
    import antenv
    mod = types.ModuleType("antenv.axon_hooks")
    holder = {"h": None}
    mod.set_axon_ntff_profile_hook = lambda h: holder.__setitem__("h", h)
    mod.get_axon_ntff_profile_hook = lambda: holder["h"]
    sys.modules["antenv.axon_hooks"] = mod
    antenv.axon_hooks = mod
    try:
        from trn_agent_boot.trn_boot import _ntff_profile_via_ctypes
        mod.set_axon_ntff_profile_hook(
            _ntff_profile_via_ctypes("/opt/axon/libaxon_pjrt.so"))
    except Exception as e:
        print(f"ntff hook unavailable: {e}")


def kernel(x, segment_ids, weight, bias, num_graphs):
    global LAST_RESULT
    import ml_dtypes
    from concourse import bass_utils

    if TRACE:
        _ensure_ntff_hook()

    f8e4 = ml_dtypes.float8_e3m4
    bf16 = ml_dtypes.bfloat16
    x = np.asarray(x, dtype=np.float32)
    seg = np.asarray(segment_ids).astype(np.int64)
    weight = np.asarray(weight, dtype=np.float32)
    bias = np.asarray(bias, dtype=np.float32)
    G = int(num_graphs)
    assert G == NUM_GRAPHS and x.shape == (N_NODES, IN_CH)

    bounds = np.searchsorted(seg, np.arange(G + 1))
    cnts = np.diff(bounds).astype(np.int64)
    m = (cnts + 7) // 8                      # slot rows per seg

    # ---- snake-pack segments into 512 bins of exactly 32 segs ----
    order = np.argsort(-m, kind="stable")
    bin_of_seg = np.empty(G, np.int64)
    local_of_seg = np.empty(G, np.int64)
    fwd = np.arange(NBIN)
    rev = fwd[::-1]
    for r in range(G // NBIN):               # 32 rounds
        rowsegs = order[r * NBIN:(r + 1) * NBIN]
        bins = fwd if r % 2 == 0 else rev
        bin_of_seg[rowsegs] = bins
        local_of_seg[rowsegs] = r
    R = np.zeros(NBIN, np.int64)
    np.add.at(R, bin_of_seg, m)
    T = int(np.ceil(R.max() / P))
    assert T * P >= R.max()

    # per-seg starting slot row within its block (assignment order per bin)
    row_start = np.zeros(G, np.int64)
    base = np.zeros(NBIN, np.int64)
    for r in range(G // NBIN):
        rowsegs = order[r * NBIN:(r + 1) * NBIN]
        b = bin_of_seg[rowsegs]
        row_start[rowsegs] = base[b]
        base[b] += m[rowsegs]

    # ---- sigma-delta quantize on e4m3 grid ----
    L = int(m.max() * 8)
    idx_in_seg = np.arange(N_NODES) - bounds[seg]
    xpad = np.zeros((G, L, IN_CH), np.float32)
    vpad = np.zeros((G, L), bool)
    xpad[seg, idx_in_seg] = x
    vpad[seg, idx_in_seg] = True
    q = _sigma_delta_fp8(xpad, vpad, f8e4)   # [G, L, F]
    del xpad, vpad
    q = q.reshape(G, L // 8, 8, IN_CH)

    # ---- scatter into per-core slabs ----
    # per slot row: seg, row index within block
    seg_rep = np.repeat(np.arange(G), m)                    # [Rtot]
    csum = np.concatenate([[0], np.cumsum(m)])
    r_in_seg = np.arange(len(seg_rep)) - csum[seg_rep]      # [Rtot]
    row_blk = row_start[seg_rep] + r_in_seg                 # block row
    bin_r = bin_of_seg[seg_rep]
    core_r = bin_r // NBLK
    blk_r = bin_r % NBLK
    g_r = row_blk // P
    p_r = row_blk % P

    HEAD = 1024
    xq_all = np.zeros((N_CORES, P, HEAD + NBLK * T * 512), f8e4)
    vals = q[seg_rep, r_in_seg]                             # [Rtot, 8, F]
    cols = (HEAD + blk_r * (T * 512) + g_r * 512)[:, None, None] + \
        (np.arange(8) * IN_CH)[None, :, None] + \
        np.arange(IN_CH)[None, None, :]
    xq_all[core_r[:, None, None], p_r[:, None, None], cols] = vals
    del q, vals, cols

    srel_all = np.full((N_CORES, P, NBLK * T), -1.0, np.float32)
    srel_all[core_r, p_r, blk_r * T + g_r] = local_of_seg[seg_rep]
    xq_all[:, :, 0:NBLK * T * 2] = np.ascontiguousarray(
        srel_all.astype(bf16)).view(np.uint8).view(f8e4)

    # ---- epilogue scale + output permutation ----
    # device row (core, grp*128 + p) holds seg with bin=core*NBLK+grp*4+p//32,
    # local=p%32
    grp = np.arange(SEGS_PER_CORE) // P
    p_of = np.arange(SEGS_PER_CORE) % P
    seg_at = np.empty((N_CORES, SEGS_PER_CORE), np.int64)
    inv = np.empty(G, np.int64)
    inv[bin_of_seg * 32 + local_of_seg] = np.arange(G)
    for c in range(N_CORES):
        bins_ = c * NBLK + grp * 4 + p_of // 32
        seg_at[c] = inv[bins_ * 32 + p_of % 32]
    sc = 1.0 / np.maximum(cnts, 1).astype(np.float32)[seg_at]  # [C, 2048]
    scale_all = np.ascontiguousarray(
        sc.reshape(N_CORES, NGROUP, P).transpose(0, 2, 1)).astype(np.float32)

    wb = np.concatenate([weight.T, bias[None]], axis=0).astype(np.float16)
    xq_all[:, :, 512:512 + 4 * NGROUP] = np.ascontiguousarray(
        scale_all).view(np.uint8).view(f8e4)
    xq_all[:, 0:IN_CH + 1, 576:576 + 2 * OUT_CH] = np.ascontiguousarray(
        wb).view(np.uint8).view(f8e4)[None]

    bkey = (T, OUT64, DEFER_SMALL, WARMUP)
    if bkey not in _BUILD_CACHE:
        _BUILD_CACHE[bkey] = _build(T)
    nc = _BUILD_CACHE[bkey]

    in_maps = [
        dict(xq=xq_all[c])
        for c in range(N_CORES)
    ]
    res = bass_utils.run_bass_kernel_spmd(
        nc, in_maps, core_ids=list(range(N_CORES)), trace=TRACE)
    LAST_RESULT = res
    if OUT64:
        dev = np.concatenate(
            [res.results[c]["out"].reshape(OUT_CH // 2, NGROUP, 2, P)
             .transpose(2, 0, 1, 3).reshape(OUT_CH, SEGS_PER_CORE).T
             for c in range(N_CORES)], axis=0)
    else:
        dev = np.concatenate(
            [res.results[c]["out"].T for c in range(N_CORES)], axis=0)
    out_full = np.empty((G, OUT_CH), np.float32)
    out_full[seg_at.reshape(-1)] = dev.astype(np.float32)
    return out_full
